# revision 53
# baseline (speedup 1.0000x reference)
"""Trainium2 Bass kernel: spiking multi-head attention (nn_MultiHeadedAttention).

Reference semantics (B=4, T=2048, DIN=100, D=512, h=8 heads, dk=64):
    q = spike(query @ Wq + bq)   (spike = (x >= 1.0) -> {0,1})
    k = spike(key @ Wk + bk);  v = spike(value @ Wv + bv)
    attn = (q @ k^T) * scale, causally masked (keep k<=q), NO softmax
    x = spike(attn @ v)
    x = x.transpose(0,1,3,2).reshape(B,T,h*dk)    # scrambled reshape
    y = spike(x @ Wo + bo)

Key facts exploited:
  * No softmax -> causal attention is LINEAR attention:
        O_t = q_t . M_t  +  intra-block tril(Q K^T) V,   M = sum_j k_j v_j^T
    The running 64x64/head state M accumulates in PSUM across 16 t-blocks.
  * The scrambled reshape maps output rows [256*h, 256*(h+1)) to exactly one
    head h, so head-parallel sharding needs NO cross-core communication.
  * Spiked tensors are {0,1}; fp16 matmuls (1 PE pass) are bit-exact for them.
  * fp32 matmuls cost 2 PE passes, each emitted as its own ~592ns
    instruction.  float32r (fp32 with the low 12 mantissa bits zeroed,
    tf32-like) runs ONE pass when the moving dim is >=256 and the hardware
    computes the exact product of the rounded operands.  All four dense
    projections (q/k/v/final) run in f32r with host-side RNE rounding of
    data+weights; the final projection's moving operand {0,1} is exact.
  * DMA issues cost ~0.6us on the issuing engine; the baseline serialized
    38 issues on Sync (~45us of dead PE at the front).  v2 issues 13 big
    transfers across the Sync/Scalar/GpSimd queues, ordered so wq/wk/kt
    piece 0 land first.

Sharding: core c -> batch b=c//2, head-group hg=c%2 (4 heads per core).

Hardware pitfalls encoded below:
  * K=64 matmuls at partition base 0 vs 64 run concurrently in disjoint PE
    row groups; concurrent writes to one PSUM bank hang the device, so the
    two parity S-tiles live in separate banks.
  * start=True zeroes a whole 2KB PSUM bank region; PSUM allocation is
    bank-granular so every tile owns its bank.
  * GPSIMD cannot read PSUM; the masked M snapshot runs on Vector.
  * f32r matmul inputs must be produced by instructions whose output dtype
    is float32r (bir verifier) -- DMA into f32r tiles and DVE f32r stores
    both qualify.
"""

import os
import numpy as np

B, T, DIN, D = 4, 2048, 100, 512
H, DK = 8, 64
NCORES = 8
HPC = 4          # heads per core
DH = HPC * DK    # 256 projected features per core
P = 128
NT = T // P      # 16 t-blocks
KC = D // P      # 4 contraction chunks of the D=512 dim
NPIECE = 4       # pipeline pieces along T

# packed-weights column offsets (4-byte columns of the [128, WPACK_W] tensor)
OFF_WQ = 0
OFF_MSK = 256
OFF_BDG = 512
OFF_WK = 768
OFF_WV = 1792
OFF_WO = 2816
OFF_BIAS = 4864
WPACK_W = 5376

_prog_cache: dict = {}
last_exec_time_ns = None

# per-projection precision: 'r' = float32r (1 PE pass), 'f' = fp32 (2 passes)
MODES = {"q": "r", "k": "r", "v": "r", "o": "r"}


def _build(scale: float, has_bk: bool, has_bv: bool, has_bo: bool, modes: dict):
    from contextlib import ExitStack

    import concourse.bass as bass
    import concourse.tile as tile
    import concourse.mybir as mybir
    from concourse import bacc
    from concourse.bass import ts
    from concourse import masks

    f32 = mybir.dt.float32
    f32r = mybir.dt.float32r
    f16 = mybir.dt.float16
    ALU = mybir.AluOpType
    AF = mybir.ActivationFunctionType
    BIG = float(2 ** 26)
    import math

    pow2_scale = scale > 0 and math.frexp(scale)[0] == 0.5

    nc = bacc.Bacc(
        "TRN2", target_bir_lowering=False, debug=False, num_devices=NCORES
    )

    # DRAM I/O.  All dense-projection operands are declared float32r; a
    # projection running in fp32 mode just bitcasts its views back to f32
    # (the host then skips rounding those sections).
    qT = nc.dram_tensor("qT", [P, T], f32r, kind="ExternalInput").ap()
    kT = nc.dram_tensor("kT", [D, T], f32r, kind="ExternalInput").ap()
    vT = nc.dram_tensor("vT", [D, T], f32r, kind="ExternalInput").ap()
    wpk = nc.dram_tensor("wpk", [P, WPACK_W], f32r, kind="ExternalInput").ap()
    # y[2m+j] = final spike block for piece m, head pair j (contiguous
    # stores; the host unscrambles the row interleave).
    y = nc.dram_tensor("y", [2 * NPIECE, P, D], f16, kind="ExternalOutput").ap()

    def mm_ops(which, lhsT, rhs):
        if modes[which] == "r":
            return lhsT, rhs
        return lhsT.bitcast(f32), rhs.bitcast(f32)

    with tile.TileContext(nc) as tc, ExitStack() as ctx:
        pool = lambda name, bufs, space="SBUF": ctx.enter_context(
            tc.tile_pool(name=name, bufs=bufs, space=space)
        )
        persist = pool("persist", 1)      # distinct tags -> own slots
        s_pool = pool("s_pool", 4)        # masked S tiles (fp16)
        t_pool = pool("t_pool", 4)        # ACT-chain temporaries
        m_pool = pool("m_pool", 2)        # masked M snapshots
        y_pool = pool("y_pool", 3)        # output staging
        pp = pool("pp", 3, "PSUM")        # projections/final/transposes
        ps = pool("ps", 1, "PSUM")        # S^T tiles (2 parity tags)
        po = pool("po", 2, "PSUM")        # O pair accumulators
        pm = pool("pm", 1, "PSUM")        # persistent M state

        def ptile(shape, dtype=f32, *, name):
            return persist.tile(shape, dtype, name=name, tag=name)

        # ---- SBUF allocations -----------------------------------------
        # Every independently-loaded region gets its OWN tile: the tile
        # framework chains DMAs writing one tile (WAW) with a ~2us
        # semaphore round-trip per link, so shared tiles serialize the
        # whole input stream.
        qt_sb = ptile([P, T], f32r, name="qt_sb")
        kt_sb = ptile([P, KC * T], f32r, name="kt_sb")
        vt_sb = ptile([P, KC * T], f32r, name="vt_sb")
        wq_t = ptile([P, DH], f32r, name="wq_t")
        mb_t = ptile([P, 2 * DH], f32r, name="mb_t")
        wk_t = ptile([P, KC * DH], f32r, name="wk_t")
        wv_t = ptile([P, KC * DH], f32r, name="wv_t")
        wob_t = ptile([P, KC * D + D], f32r, name="wob_t")
        wq_sb = wq_t[:, :]
        msk_sb = mb_t[:, 0:DH].bitcast(f32)
        bdg_sb = mb_t[:, DH : 2 * DH].bitcast(f32)
        wk_sb = [wk_t[:, 256 * c : 256 * (c + 1)] for c in range(KC)]
        wv_sb = [wv_t[:, 256 * c : 256 * (c + 1)] for c in range(KC)]
        wo_sb = [wob_t[:, 512 * c : 512 * (c + 1)] for c in range(KC)]
        bias_sb = wob_t[:, KC * D : KC * D + D]
        ones_sb = ptile([1, D], f32r, name="ones_sb")
        idt_sb = ptile([P, P], f16, name="idt_sb")
        # qs/ks: spiked projections, d-major [dk, T]; tile i holds heads
        # 2i (parts 0:64) and 2i+1 (parts 64:128).  fp16: {0,1} and the
        # integer M state (<= 2048 < 2^11) are exact, 1 PE pass.
        qs = [ptile([P, T], f16, name=f"qs{i}") for i in range(2)]
        ks = [ptile([P, T], f16, name=f"ks{i}") for i in range(2)]
        # vkn: t-major spiked v for all 4 heads (cols 256t+64*hl), fp16.
        vkn = ptile([P, DH * NT], f16, name="vkn")
        # kn: t-major spiked k via PE transpose of ks, pair-major:
        # cols 256t + 128*pair + 64*(hl%2)
        kn = ptile([P, DH * NT], f16, name="kn")
        # xs: spiked attention output, xs[p, 256*t_blk + 128*pair + 64*par
        # + d]; f32r so the final projection consumes it in one PE pass
        # ({0,1} exact), contiguous per (t_blk, pair) for both the DVE
        # store and the final-proj weight load.
        xs = ptile([P, 256 * NT], f32r, name="xs")

        # ---- loads ----------------------------------------------------
        # Sync carries the kproj critical path (kt piece 0 per chunk, wk,
        # then qT), Scalar carries wv/vt piece 0 + wo, GpSimd the vt bulk.
        # Distinct dst tiles keep every queue's transfers streaming
        # back-to-back with no cross-transfer semaphore links.
        PW = T // NPIECE
        # Arrival-deadline schedule.  Big 3D piece transfers (all four
        # 128-row chunks in one ~1MB issue) beat per-chunk 0.25MB issues
        # (~1.3us fixed cost each).  Sync carries the k-side critical
        # path; Scalar only 4 early issues (its ACT work starts ~13us);
        # the vt bulk rides GpSimd behind a gate-copy so it cannot steal
        # ring bandwidth from the prefix.  make_identity is emitted
        # before the gate so the transposes' identity tile exists early.
        kt_r = kt_sb[:, :].rearrange("p (c t) -> p c t", c=KC)
        vt_r = vt_sb[:, :].rearrange("p (c t) -> p c t", c=KC)
        kT_r = kT[:, :].rearrange("(c p) t -> p c t", p=P)
        vT_r = vT[:, :].rearrange("(c p) t -> p c t", p=P)
        nc.sync.dma_start(
            out=wq_t[:, :], in_=wpk[:, OFF_WQ : OFF_WQ + DH]
        )
        nc.sync.dma_start(out=wk_t[:, :], in_=wpk[:, OFF_WK:OFF_WV])
        nc.sync.dma_start(
            out=kt_r[:, :, ts(0, PW)], in_=kT_r[:, :, ts(0, PW)]
        )
        nc.sync.dma_start(out=mb_t[:, :], in_=wpk[:, OFF_MSK:OFF_WK])
        nc.sync.dma_start(out=wob_t[:, :], in_=wpk[:, OFF_WO:WPACK_W])
        for pc in range(1, NPIECE):
            nc.sync.dma_start(
                out=kt_r[:, :, ts(pc, PW)], in_=kT_r[:, :, ts(pc, PW)]
            )
        nc.scalar.dma_start(out=qt_sb[:, 0:512], in_=qT[:, 0:512])
        nc.scalar.dma_start(out=wv_t[:, :], in_=wpk[:, OFF_WV:OFF_WO])
        nc.scalar.dma_start(
            out=vt_r[:, :, ts(0, PW)], in_=vT_r[:, :, ts(0, PW)]
        )
        nc.scalar.dma_start(out=qt_sb[:, 512:T], in_=qT[:, 512:T])
        nc.vector.memset(ones_sb[:, :].bitcast(f32), 1.0)
        masks.make_identity(nc, idt_sb[:, :])
        # gate: the copy reads vt piece 0 (RAW), so later GpSimd
        # instructions (FIFO) wait for the prefix before the bulk pull.
        gate_sb = ptile([1, 1], f32, name="gate_sb")
        nc.gpsimd.tensor_copy(
            gate_sb[:, :], vt_sb[0:1, PW - 1 : PW].bitcast(f32)
        )
        for pc in range(1, NPIECE):
            nc.gpsimd.dma_start(
                out=vt_r[:, :, ts(pc, PW)], in_=vT_r[:, :, ts(pc, PW)]
            )

        def spike_act(out_ap, in_ap, nm):
            """out = (in >= 1.0) via two exact Relu ops on the ACT engine."""
            tmp = t_pool.tile(list(out_ap.shape), f32, name=f"tmp_{nm}")
            nc.scalar.activation(tmp[:, :], in_ap, AF.Relu, bias=1.0, scale=-1.0)
            nc.scalar.activation(out_ap, tmp[:, :], AF.Relu, bias=1.0, scale=-BIG)

        # ---- qs projection (only needs qt + wq) ------------------------
        def qproj(chunks):
            for ch in chunks:
                for half in range(2):
                    pt = pp.tile([P, 512], f32, name="pt", tag="pt")
                    lhsT, rhs = mm_ops(
                        "q", wq_sb[:, ts(half, P)], qt_sb[:, ts(ch, 512)]
                    )
                    nc.tensor.matmul(
                        pt[:, :], lhsT=lhsT, rhs=rhs, start=True, stop=True
                    )
                    spike_act(qs[half][:, ts(ch, 512)], pt[:, :], "q")

        # ---- pipelined: per piece, ks chunk -> vkn blocks -> attention -
        pm_t = pm.tile([P, DH], f32, name="pm_t")

        def ks_chunk(ch):
            for half in range(2):
                pt = pp.tile([P, 512], f32, name="pt", tag="pt")
                for c in range(KC):
                    lhsT, rhs = mm_ops(
                        "k",
                        wk_sb[c][:, ts(half, P)],
                        kt_sb[:, c * T :][:, ts(ch, 512)],
                    )
                    nc.tensor.matmul(
                        pt[:, :],
                        lhsT=lhsT,
                        rhs=rhs,
                        start=(c == 0),
                        stop=(c == KC - 1) and not has_bk,
                    )
                if has_bk:
                    nc.tensor.matmul(
                        pt[:, :],
                        lhsT=bias_sb[0:1, ts(half, P)],
                        rhs=ones_sb[0:1, 0:512],
                        start=False,
                        stop=True,
                    )
                spike_act(ks[half][:, ts(ch, 512)], pt[:, :], "k")
            # t-major spiked K for this chunk's 4 blocks via PE transpose;
            # a [128,128] head-pair tile transpose lands exactly in the
            # pair-major layout the M-update wants.  (A DMA-xbar transpose
            # is bit-exact in isolation but showed ~100 extra spike flips
            # when overlapped with the input loads, so it stays on the PE.)
            for tt in range(4 * ch, 4 * ch + 4):
                for pr in range(2):
                    tp = pp.tile([P, P], f16, name="tp", tag="pt")
                    nc.tensor.transpose(
                        tp[:, :], ks[pr][:, ts(tt, P)], idt_sb[:, :]
                    )
                    nc.vector.tensor_copy(
                        kn[:, DH * tt + P * pr :][:, 0:P], tp[:, :]
                    )

        def vkn_block(tt):
            pt = pp.tile([P, 512], f32, name="pt", tag="pt")
            for c in range(KC):
                lhsT, rhs = mm_ops(
                    "v", vt_sb[:, c * T :][:, ts(tt, P)], wv_sb[c][:, :]
                )
                nc.tensor.matmul(
                    pt[:, 0:DH],
                    lhsT=lhsT,
                    rhs=rhs,
                    start=(c == 0),
                    stop=(c == KC - 1) and not has_bv,
                )
            if has_bv:
                nc.tensor.matmul(
                    pt[:, 0:DH],
                    lhsT=ones_sb[0:1, 0:P],
                    rhs=bias_sb[1:2, 0:DH],
                    start=False,
                    stop=True,
                )
            nc.vector.tensor_scalar(
                vkn[:, ts(tt, DH)], pt[:, 0:DH], 1.0, None, ALU.is_ge
            )

        def attn_block(tt):
            if tt > 0:
                # masked snapshot M_(<tt): zero the cross-head 64x64 blocks
                # so the pair O-inter matmul can contract over all 128
                # partition rows at once.
                m_sb = m_pool.tile([P, DH], f16, name="m_sb")
                nc.vector.tensor_tensor(
                    m_sb[:, :], pm_t[:, :], bdg_sb[:, :], op=ALU.mult
                )
            s_ps = [
                ps.tile([P, DH], f32, name=f"s_ps{par}", tag=f"s_ps{par}")
                for par in range(2)
            ]
            for hl in range(HPC):
                par, idx = hl % 2, hl // 2
                rows = slice(64 * par, 64 * par + 64)
                nc.tensor.matmul(
                    s_ps[par][:, ts(idx, P)],
                    lhsT=ks[idx][rows, ts(tt, P)],
                    rhs=qs[idx][rows, ts(tt, P)],
                    start=True,
                    stop=True,
                )
            s_sb = [
                s_pool.tile([P, DH], f16, name=f"s_sb{par}", tag=f"s_sb{par}")
                for par in range(2)
            ]
            for par in range(2):
                nc.vector.tensor_tensor(
                    s_sb[par][:, :], s_ps[par][:, :], msk_sb[:, :], op=ALU.mult
                )
            # O pair tiles: cols 0:64 head 2*idx, 64:128 head 2*idx+1.
            o_ps = [po.tile([P, P], f32, name="o_ps") for _ in range(2)]
            for idx in range(2):
                if tt > 0:
                    nc.tensor.matmul(
                        o_ps[idx][:, :],
                        lhsT=qs[idx][:, ts(tt, P)],
                        rhs=m_sb[:, ts(idx, P)],
                        start=True,
                        stop=False,
                        skip_group_check=True,
                    )
            for hl in range(HPC):
                par, idx = hl % 2, hl // 2
                nc.tensor.matmul(
                    o_ps[idx][:, ts(par, 64)],
                    lhsT=s_sb[par][:, ts(idx, P)],
                    rhs=vkn[:, DH * tt + 64 * hl :][:, 0:64],
                    start=(tt == 0),
                    stop=(par == 1),
                    skip_group_check=True,
                )
            # M += K_pair^T V_pair: one K=128,N=128 matmul per head pair;
            # cross 64x64 blocks hold garbage, masked out at snapshot time.
            for pr in range(2):
                nc.tensor.matmul(
                    pm_t[:, ts(pr, P)],
                    lhsT=kn[:, DH * tt + P * pr :][:, 0:P],
                    rhs=vkn[:, DH * tt + P * pr :][:, 0:P],
                    start=(tt == 0 and pr == 0),
                    stop=(pr == 1),
                    skip_group_check=True,
                )
            # x = spike(scale * O).  O is integer, so for power-of-two
            # scale this is exactly (O >= 1/scale): one DVE op straight
            # from PSUM into the f32r xs tile.  Otherwise fall back to the
            # exact relu(1 - scale*O) <= 0 two-op chain.
            for idx in range(2):
                if pow2_scale:
                    nc.vector.tensor_scalar(
                        xs[:, 256 * tt + 128 * idx :][:, 0:P],
                        o_ps[idx][:, :],
                        float(1.0 / scale),
                        None,
                        ALU.is_ge,
                    )
                else:
                    xtmp = t_pool.tile([P, P], f32, name="xtmp")
                    nc.scalar.activation(
                        xtmp[:, :], o_ps[idx][:, :], AF.Relu,
                        bias=1.0, scale=-float(scale),
                    )
                    nc.vector.tensor_scalar(
                        xs[:, 256 * tt + 128 * idx :][:, 0:P],
                        xtmp[:, :],
                        0.0,
                        None,
                        ALU.is_le,
                    )

        def proj_piece(pc):
            ks_chunk(pc)
            for tt in range(4 * pc, 4 * pc + 4):
                vkn_block(tt)
        # Final projection per piece: output rows r with r%4 == m contract
        # only over attention piece m (X[r, f] = x_att[t=512*(r%4)+f,
        # d=r//4]).  A head pair's 128 rows are one contiguous xs block.

        def final_acc(yps, m, cc):
            for j in range(2):
                lhsT, rhs = mm_ops(
                    "o",
                    xs[:, 256 * (4 * m + cc) + 128 * j :][:, 0:P],
                    wo_sb[cc][:, :],
                )
                nc.tensor.matmul(
                    yps[j][:, :],
                    lhsT=lhsT,
                    rhs=rhs,
                    start=(cc == 0),
                    stop=(cc == KC - 1) and not has_bo,
                )

        def final_finish(yps, m):
            for j in range(2):  # head pair: heads 2j, 2j+1
                if has_bo:
                    nc.tensor.matmul(
                        yps[j][:, :],
                        lhsT=ones_sb[0:1, 0:P],
                        rhs=bias_sb[2:3, :],
                        start=False,
                        stop=True,
                    )
                y_sb = y_pool.tile([P, D], f16, name="y_sb")
                nc.vector.tensor_scalar(
                    y_sb[:, :], yps[j][:, :], 1.0, None, ALU.is_ge
                )
                nc.gpsimd.dma_start(out=y[2 * m + j], in_=y_sb[:, :])

        def final_piece(m):
            yps = [pp.tile([P, 512], f32, name="pt", tag="pt") for _ in range(2)]
            for cc in range(KC):
                final_acc(yps, m, cc)
            final_finish(yps, m)

        # Emission order tuned so the Tensor queue never stalls on a
        # transfer that is still behind others in a DMA queue: piece-0
        # work (smallest data prefix) first, attention starts before the
        # remaining qs chunks, wo arrives (Scalar queue) by final_piece(0).
        qproj([0])
        proj_piece(0)
        for tt in range(0, 4):
            attn_block(tt)
        qproj([1])
        proj_piece(1)
        final_piece(0)
        qproj([2])
        proj_piece(2)
        for tt in range(4, 8):
            attn_block(tt)
        final_piece(1)
        qproj([3])
        proj_piece(3)
        # pieces 2/3: no projection work remains, so the pp "pt" bufs are
        # free to hold the final-projection accumulators across the
        # attention blocks -- only spike+store remain after the last block.
        yps2 = [pp.tile([P, 512], f32, name="pt", tag="pt") for _ in range(2)]
        for tt in range(8, 12):
            attn_block(tt)
            final_acc(yps2, 2, tt % 4)
        final_finish(yps2, 2)
        yps3 = [pp.tile([P, 512], f32, name="pt", tag="pt") for _ in range(2)]
        for tt in range(12, 16):
            attn_block(tt)
            final_acc(yps3, 3, tt % 4)
        final_finish(yps3, 3)

    nc.compile()
    return nc


def _get_prog(scale, has_bk, has_bv, has_bo):
    key = (scale, has_bk, has_bv, has_bo, tuple(sorted(MODES.items())))
    if key not in _prog_cache:
        _prog_cache[key] = _build(scale, has_bk, has_bv, has_bo, MODES)
    return _prog_cache[key]


def _rne12(x):
    """Round fp32 -> float32r (11 explicit mantissa bits, RNE)."""
    u = np.ascontiguousarray(x, dtype=np.float32).view(np.uint32).astype(np.uint64)
    lsb = (u >> 12) & 1
    u = (u + 0x7FF + lsb) & 0xFFFFF000
    return u.astype(np.uint32).view(np.float32)


def _pack_weights(Wq, bq, Wk, bk, Wv, bv, Wo, bo, cs):
    r = lambda which, x: _rne12(x) if MODES[which] == "r" else np.float32(x)
    wpk = np.zeros((P, WPACK_W), np.float32)
    wpk[:DIN, OFF_WQ : OFF_WQ + DH] = r("q", Wq[:, cs])
    wpk[DIN, OFF_WQ : OFF_WQ + DH] = r("q", bq[cs])
    wpk[:, OFF_MSK : OFF_MSK + DH] = np.tile(
        np.triu(np.ones((P, P), np.float32)), (1, 2)
    )
    bdg = np.zeros((P, DH), np.float32)
    for pr in range(2):
        for par in range(2):
            sl = slice(64 * par, 64 * par + 64)
            bdg[sl, 128 * pr + 64 * par : 128 * pr + 64 * par + 64] = 1.0
    wpk[:, OFF_BDG : OFF_BDG + DH] = bdg
    for c in range(KC):
        wpk[:, OFF_WK + 256 * c : OFF_WK + 256 * (c + 1)] = r(
            "k", Wk[128 * c : 128 * (c + 1), cs]
        )
        wpk[:, OFF_WV + 256 * c : OFF_WV + 256 * (c + 1)] = r(
            "v", Wv[128 * c : 128 * (c + 1), cs]
        )
        wpk[:, OFF_WO + 512 * c : OFF_WO + 512 * (c + 1)] = r(
            "o", Wo[128 * c : 128 * (c + 1), :]
        )
    wpk[0, OFF_BIAS : OFF_BIAS + DH] = r("k", bk[cs])
    wpk[1, OFF_BIAS : OFF_BIAS + DH] = r("v", bv[cs])
    wpk[2, OFF_BIAS : OFF_BIAS + D] = r("o", bo)
    return wpk


def kernel(**inputs) -> np.ndarray:
    global last_exec_time_ns
    from concourse.bass_utils import run_bass_kernel_spmd

    g = lambda n: np.asarray(inputs[n], dtype=np.float32)
    query, key, value = g("query"), g("key"), g("value")
    Wq, bq, Wk, bk = g("Wq"), g("bq"), g("Wk"), g("bk")
    Wv, bv, Wo, bo = g("Wv"), g("bv"), g("Wo"), g("bo")
    scale = float(np.asarray(inputs["scale"], dtype=np.float32).reshape(-1)[0])

    has_bk, has_bv, has_bo = (bool(np.any(x)) for x in (bk, bv, bo))
    prog = _get_prog(scale, has_bk, has_bv, has_bo)

    rd = lambda which, x: _rne12(x) if MODES[which] == "r" else np.ascontiguousarray(x, np.float32)
    in_maps = []
    for c in range(NCORES):
        b, hg = divmod(c, 2)
        cs = slice(DH * hg, DH * (hg + 1))
        qTa = np.zeros((P, T), np.float32)
        qTa[:DIN] = rd("q", query[b].T)
        qTa[DIN] = 1.0
        in_maps.append(
            {
                "qT": qTa,
                "kT": rd("k", key[b].T),
                "vT": rd("v", value[b].T),
                "wpk": _pack_weights(Wq, bq, Wk, bk, Wv, bv, Wo, bo, cs),
            }
        )

    trace = os.environ.get("BASS_TRACE", "") not in ("", "0")
    res = run_bass_kernel_spmd(
        prog, in_maps, core_ids=list(range(NCORES)), trace=trace
    )
    last_exec_time_ns = res.exec_time_ns
    if res.exec_time_ns is not None:
        print(f"HW exec time: {res.exec_time_ns} ns")

    # y[2m+j, 64*sub + i, :] -> full row 256*(2j+sub) + m + 4i
    mi, pi = np.meshgrid(np.arange(2 * NPIECE), np.arange(P), indexing="ij")
    m, j, sub, i = mi // 2, mi % 2, pi // 64, pi % 64
    rows = (256 * (2 * j + sub) + m + 4 * i).ravel()
    inv = np.empty(1024, np.int64)
    inv[rows] = np.arange(1024)
    out = np.empty((B, T, D), np.float32)
    for c in range(NCORES):
        b, hg = divmod(c, 2)
        yc = res.results[c]["y"].reshape(1024, D)
        out[b, 1024 * hg : 1024 * (hg + 1)] = yc[inv].astype(np.float32)
    return out


# revision 57
# speedup vs baseline: 1.0163x; 1.0163x over previous
"""Trainium2 Bass kernel: spiking multi-head attention (nn_MultiHeadedAttention).

Reference semantics (B=4, T=2048, DIN=100, D=512, h=8 heads, dk=64):
    q = spike(query @ Wq + bq)   (spike = (x >= 1.0) -> {0,1})
    k = spike(key @ Wk + bk);  v = spike(value @ Wv + bv)
    attn = (q @ k^T) * scale, causally masked (keep k<=q), NO softmax
    x = spike(attn @ v)
    x = x.transpose(0,1,3,2).reshape(B,T,h*dk)    # scrambled reshape
    y = spike(x @ Wo + bo)

Key facts exploited:
  * No softmax -> causal attention is LINEAR attention:
        O_t = q_t . M_t  +  intra-block tril(Q K^T) V,   M = sum_j k_j v_j^T
    The running 64x64/head state M accumulates in PSUM across 16 t-blocks.
  * The scrambled reshape maps output rows [256*h, 256*(h+1)) to exactly one
    head h, so head-parallel sharding needs NO cross-core communication.
  * Spiked tensors are {0,1}; fp16 matmuls (1 PE pass) are bit-exact for them.
  * fp32 matmuls cost 2 PE passes, each emitted as its own ~592ns
    instruction.  float32r (fp32 with the low 12 mantissa bits zeroed,
    tf32-like) runs ONE pass when the moving dim is >=256 and the hardware
    computes the exact product of the rounded operands.  All four dense
    projections (q/k/v/final) run in f32r with host-side RNE rounding of
    data+weights; the final projection's moving operand {0,1} is exact.
  * DMA issues cost ~0.6us on the issuing engine; the baseline serialized
    38 issues on Sync (~45us of dead PE at the front).  v2 issues 13 big
    transfers across the Sync/Scalar/GpSimd queues, ordered so wq/wk/kt
    piece 0 land first.

Sharding: core c -> batch b=c//2, head-group hg=c%2 (4 heads per core).

Hardware pitfalls encoded below:
  * K=64 matmuls at partition base 0 vs 64 run concurrently in disjoint PE
    row groups; concurrent writes to one PSUM bank hang the device, so the
    two parity S-tiles live in separate banks.
  * start=True zeroes a whole 2KB PSUM bank region; PSUM allocation is
    bank-granular so every tile owns its bank.
  * GPSIMD cannot read PSUM; the masked M snapshot runs on Vector.
  * f32r matmul inputs must be produced by instructions whose output dtype
    is float32r (bir verifier) -- DMA into f32r tiles and DVE f32r stores
    both qualify.
"""

import os
import numpy as np

B, T, DIN, D = 4, 2048, 100, 512
H, DK = 8, 64
NCORES = 8
HPC = 4          # heads per core
DH = HPC * DK    # 256 projected features per core
P = 128
NT = T // P      # 16 t-blocks
KC = D // P      # 4 contraction chunks of the D=512 dim
NPIECE = 4       # pipeline pieces along T

# packed-weights column offsets (4-byte columns of the [128, WPACK_W] tensor)
OFF_WQ = 0
OFF_MSK = 256
OFF_BDG = 512
OFF_WK = 768
OFF_WV = 1792
OFF_WO = 2816
OFF_BIAS = 4864
WPACK_W = 5376

_prog_cache: dict = {}
last_exec_time_ns = None

# per-projection precision: 'r' = float32r (1 PE pass), 'f' = fp32 (2 passes)
MODES = {"q": "r", "k": "r", "v": "r", "o": "r"}


def _build(scale: float, has_bk: bool, has_bv: bool, has_bo: bool, modes: dict):
    from contextlib import ExitStack

    import concourse.bass as bass
    import concourse.tile as tile
    import concourse.mybir as mybir
    from concourse import bacc
    from concourse.bass import ts
    from concourse import masks

    f32 = mybir.dt.float32
    f32r = mybir.dt.float32r
    f16 = mybir.dt.float16
    ALU = mybir.AluOpType
    AF = mybir.ActivationFunctionType
    BIG = float(2 ** 26)
    import math

    pow2_scale = scale > 0 and math.frexp(scale)[0] == 0.5

    nc = bacc.Bacc(
        "TRN2", target_bir_lowering=False, debug=False, num_devices=NCORES
    )

    # DRAM I/O.  All dense-projection operands are declared float32r; a
    # projection running in fp32 mode just bitcasts its views back to f32
    # (the host then skips rounding those sections).
    qT = nc.dram_tensor("qT", [P, T], f32r, kind="ExternalInput").ap()
    kT = nc.dram_tensor("kT", [D, T], f32r, kind="ExternalInput").ap()
    vT = nc.dram_tensor("vT", [D, T], f32r, kind="ExternalInput").ap()
    wpk = nc.dram_tensor("wpk", [P, WPACK_W], f32r, kind="ExternalInput").ap()
    # y[2m+j] = final spike block for piece m, head pair j (contiguous
    # stores; the host unscrambles the row interleave).
    y = nc.dram_tensor("y", [2 * NPIECE, P, D], f16, kind="ExternalOutput").ap()

    def mm_ops(which, lhsT, rhs):
        if modes[which] == "r":
            return lhsT, rhs
        return lhsT.bitcast(f32), rhs.bitcast(f32)

    with tile.TileContext(nc) as tc, ExitStack() as ctx:
        pool = lambda name, bufs, space="SBUF": ctx.enter_context(
            tc.tile_pool(name=name, bufs=bufs, space=space)
        )
        persist = pool("persist", 1)      # distinct tags -> own slots
        s_pool = pool("s_pool", 4)        # masked S tiles (fp16)
        t_pool = pool("t_pool", 4)        # ACT-chain temporaries
        m_pool = pool("m_pool", 2)        # masked M snapshots
        y_pool = pool("y_pool", 3)        # output staging
        pp = pool("pp", 3, "PSUM")        # projections/final/transposes
        ps = pool("ps", 1, "PSUM")        # S^T tiles (2 parity tags)
        po = pool("po", 2, "PSUM")        # O pair accumulators
        pm = pool("pm", 1, "PSUM")        # persistent M state

        def ptile(shape, dtype=f32, *, name):
            return persist.tile(shape, dtype, name=name, tag=name)

        # ---- SBUF allocations -----------------------------------------
        # Every independently-loaded region gets its OWN tile: the tile
        # framework chains DMAs writing one tile (WAW) with a ~2us
        # semaphore round-trip per link, so shared tiles serialize the
        # whole input stream.
        qt_sb = ptile([P, T], f32r, name="qt_sb")
        kt_sb = [ptile([P, T], f32r, name=f"kt_sb{c}") for c in range(KC)]
        vt_sb = [ptile([P, T], f32r, name=f"vt_sb{c}") for c in range(KC)]
        wq_t = ptile([P, DH], f32r, name="wq_t")
        mb_t = ptile([P, 2 * DH], f32r, name="mb_t")
        wk_t = ptile([P, KC * DH], f32r, name="wk_t")
        wv_t = ptile([P, KC * DH], f32r, name="wv_t")
        wob_t = ptile([P, KC * D + D], f32r, name="wob_t")
        wq_sb = wq_t[:, :]
        msk_sb = mb_t[:, 0:DH].bitcast(f32)
        bdg_sb = mb_t[:, DH : 2 * DH].bitcast(f32)
        wk_sb = [wk_t[:, 256 * c : 256 * (c + 1)] for c in range(KC)]
        wv_sb = [wv_t[:, 256 * c : 256 * (c + 1)] for c in range(KC)]
        wo_sb = [wob_t[:, 512 * c : 512 * (c + 1)] for c in range(KC)]
        bias_sb = wob_t[:, KC * D : KC * D + D]
        ones_sb = ptile([1, D], f32r, name="ones_sb")
        idt_sb = ptile([P, P], f16, name="idt_sb")
        # qs/ks: spiked projections, d-major [dk, T]; tile i holds heads
        # 2i (parts 0:64) and 2i+1 (parts 64:128).  fp16: {0,1} and the
        # integer M state (<= 2048 < 2^11) are exact, 1 PE pass.
        qs = [ptile([P, T], f16, name=f"qs{i}") for i in range(2)]
        ks = [ptile([P, T], f16, name=f"ks{i}") for i in range(2)]
        # vkn: t-major spiked v for all 4 heads (cols 256t+64*hl), fp16.
        vkn = ptile([P, DH * NT], f16, name="vkn")
        # kn: t-major spiked k via PE transpose of ks, pair-major:
        # cols 256t + 128*pair + 64*(hl%2)
        kn = ptile([P, DH * NT], f16, name="kn")
        # xs: spiked attention output, xs[p, 256*t_blk + 128*pair + 64*par
        # + d]; f32r so the final projection consumes it in one PE pass
        # ({0,1} exact), contiguous per (t_blk, pair) for both the DVE
        # store and the final-proj weight load.
        xs = ptile([P, 256 * NT], f32r, name="xs")

        # ---- loads ----------------------------------------------------
        # Sync carries the kproj critical path (kt piece 0 per chunk, wk,
        # then qT), Scalar carries wv/vt piece 0 + wo, GpSimd the vt bulk.
        # Distinct dst tiles keep every queue's transfers streaming
        # back-to-back with no cross-transfer semaphore links.
        PW = T // NPIECE
        # Arrival-deadline schedule.  Big 3D piece transfers (all four
        # 128-row chunks in one ~1MB issue) beat per-chunk 0.25MB issues
        # (~1.3us fixed cost each).  Sync carries the k-side critical
        # path; Scalar only 4 early issues (its ACT work starts ~13us);
        # the vt bulk rides GpSimd behind a gate-copy so it cannot steal
        # ring bandwidth from the prefix.  make_identity is emitted
        # before the gate so the transposes' identity tile exists early.
        nc.sync.dma_start(
            out=wq_t[:, :], in_=wpk[:, OFF_WQ : OFF_WQ + DH]
        )
        nc.sync.dma_start(out=wk_t[:, :], in_=wpk[:, OFF_WK:OFF_WV])
        nc.sync.dma_start(out=kt_sb[0][:, 0:PW], in_=kT[ts(0, P), 0:PW])
        nc.sync.dma_start(out=kt_sb[2][:, 0:PW], in_=kT[ts(2, P), 0:PW])
        nc.sync.dma_start(out=mb_t[:, :], in_=wpk[:, OFF_MSK:OFF_WK])
        for pc in range(1, NPIECE):
            for c in range(KC):
                nc.sync.dma_start(
                    out=kt_sb[c][:, ts(pc, PW)], in_=kT[ts(c, P), ts(pc, PW)]
                )
            if pc == 1:
                nc.sync.dma_start(
                    out=wob_t[:, :], in_=wpk[:, OFF_WO:WPACK_W]
                )
        nc.scalar.dma_start(out=qt_sb[:, 0:512], in_=qT[:, 0:512])
        nc.scalar.dma_start(out=kt_sb[1][:, 0:PW], in_=kT[ts(1, P), 0:PW])
        nc.scalar.dma_start(out=wv_t[:, :], in_=wpk[:, OFF_WV:OFF_WO])
        nc.scalar.dma_start(out=kt_sb[3][:, 0:PW], in_=kT[ts(3, P), 0:PW])
        for c in range(KC):
            nc.scalar.dma_start(out=vt_sb[c][:, 0:PW], in_=vT[ts(c, P), 0:PW])
        nc.scalar.dma_start(out=qt_sb[:, 512:T], in_=qT[:, 512:T])
        nc.vector.memset(ones_sb[:, :].bitcast(f32), 1.0)
        masks.make_identity(nc, idt_sb[:, :])
        # gate: the copy reads vt piece 0 (RAW), so later GpSimd
        # instructions (FIFO) wait for the prefix before the bulk pull.
        # Emitted after make_identity so the transposes' identity tile is
        # built before the gate blocks the GpSimd queue.
        gate_sb = ptile([1, 1], f32, name="gate_sb")
        nc.gpsimd.tensor_copy(
            gate_sb[:, :], vt_sb[KC - 1][0:1, PW - 1 : PW].bitcast(f32)
        )
        for pc in range(1, NPIECE):
            for c in range(KC):
                nc.gpsimd.dma_start(
                    out=vt_sb[c][:, ts(pc, PW)], in_=vT[ts(c, P), ts(pc, PW)]
                )

        def spike_act(out_ap, in_ap, nm):
            """out = (in >= 1.0) via two exact Relu ops on the ACT engine."""
            tmp = t_pool.tile(list(out_ap.shape), f32, name=f"tmp_{nm}")
            nc.scalar.activation(tmp[:, :], in_ap, AF.Relu, bias=1.0, scale=-1.0)
            nc.scalar.activation(out_ap, tmp[:, :], AF.Relu, bias=1.0, scale=-BIG)

        # ---- qs projection (only needs qt + wq) ------------------------
        def qproj(chunks):
            for ch in chunks:
                for half in range(2):
                    pt = pp.tile([P, 512], f32, name="pt", tag="pt")
                    lhsT, rhs = mm_ops(
                        "q", wq_sb[:, ts(half, P)], qt_sb[:, ts(ch, 512)]
                    )
                    nc.tensor.matmul(
                        pt[:, :], lhsT=lhsT, rhs=rhs, start=True, stop=True
                    )
                    spike_act(qs[half][:, ts(ch, 512)], pt[:, :], "q")

        # ---- pipelined: per piece, ks chunk -> vkn blocks -> attention -
        pm_t = pm.tile([P, DH], f32, name="pm_t")

        def ks_chunk(ch):
            for half in range(2):
                pt = pp.tile([P, 512], f32, name="pt", tag="pt")
                for c in range(KC):
                    lhsT, rhs = mm_ops(
                        "k", wk_sb[c][:, ts(half, P)], kt_sb[c][:, ts(ch, 512)]
                    )
                    nc.tensor.matmul(
                        pt[:, :],
                        lhsT=lhsT,
                        rhs=rhs,
                        start=(c == 0),
                        stop=(c == KC - 1) and not has_bk,
                    )
                if has_bk:
                    nc.tensor.matmul(
                        pt[:, :],
                        lhsT=bias_sb[0:1, ts(half, P)],
                        rhs=ones_sb[0:1, 0:512],
                        start=False,
                        stop=True,
                    )
                spike_act(ks[half][:, ts(ch, 512)], pt[:, :], "k")
            # t-major spiked K for this chunk's 4 blocks via PE transpose;
            # a [128,128] head-pair tile transpose lands exactly in the
            # pair-major layout the M-update wants.  (A DMA-xbar transpose
            # is bit-exact in isolation but showed ~100 extra spike flips
            # when overlapped with the input loads, so it stays on the PE.)
            for tt in range(4 * ch, 4 * ch + 4):
                for pr in range(2):
                    tp = pp.tile([P, P], f16, name="tp", tag="pt")
                    nc.tensor.transpose(
                        tp[:, :], ks[pr][:, ts(tt, P)], idt_sb[:, :]
                    )
                    nc.vector.tensor_copy(
                        kn[:, DH * tt + P * pr :][:, 0:P], tp[:, :]
                    )

        def vkn_block(tt):
            pt = pp.tile([P, 512], f32, name="pt", tag="pt")
            for c in range(KC):
                lhsT, rhs = mm_ops(
                    "v", vt_sb[c][:, ts(tt, P)], wv_sb[c][:, :]
                )
                nc.tensor.matmul(
                    pt[:, 0:DH],
                    lhsT=lhsT,
                    rhs=rhs,
                    start=(c == 0),
                    stop=(c == KC - 1) and not has_bv,
                )
            if has_bv:
                nc.tensor.matmul(
                    pt[:, 0:DH],
                    lhsT=ones_sb[0:1, 0:P],
                    rhs=bias_sb[1:2, 0:DH],
                    start=False,
                    stop=True,
                )
            nc.vector.tensor_scalar(
                vkn[:, ts(tt, DH)], pt[:, 0:DH], 1.0, None, ALU.is_ge
            )

        def attn_block(tt):
            if tt > 0:
                # masked snapshot M_(<tt): zero the cross-head 64x64 blocks
                # so the pair O-inter matmul can contract over all 128
                # partition rows at once.
                m_sb = m_pool.tile([P, DH], f16, name="m_sb")
                nc.vector.tensor_tensor(
                    m_sb[:, :], pm_t[:, :], bdg_sb[:, :], op=ALU.mult
                )
            s_ps = [
                ps.tile([P, DH], f32, name=f"s_ps{par}", tag=f"s_ps{par}")
                for par in range(2)
            ]
            for hl in range(HPC):
                par, idx = hl % 2, hl // 2
                rows = slice(64 * par, 64 * par + 64)
                nc.tensor.matmul(
                    s_ps[par][:, ts(idx, P)],
                    lhsT=ks[idx][rows, ts(tt, P)],
                    rhs=qs[idx][rows, ts(tt, P)],
                    start=True,
                    stop=True,
                )
            s_sb = [
                s_pool.tile([P, DH], f16, name=f"s_sb{par}", tag=f"s_sb{par}")
                for par in range(2)
            ]
            for par in range(2):
                nc.vector.tensor_tensor(
                    s_sb[par][:, :], s_ps[par][:, :], msk_sb[:, :], op=ALU.mult
                )
            # O pair tiles: cols 0:64 head 2*idx, 64:128 head 2*idx+1.
            o_ps = [po.tile([P, P], f32, name="o_ps") for _ in range(2)]
            for idx in range(2):
                if tt > 0:
                    nc.tensor.matmul(
                        o_ps[idx][:, :],
                        lhsT=qs[idx][:, ts(tt, P)],
                        rhs=m_sb[:, ts(idx, P)],
                        start=True,
                        stop=False,
                        skip_group_check=True,
                    )
            for hl in range(HPC):
                par, idx = hl % 2, hl // 2
                nc.tensor.matmul(
                    o_ps[idx][:, ts(par, 64)],
                    lhsT=s_sb[par][:, ts(idx, P)],
                    rhs=vkn[:, DH * tt + 64 * hl :][:, 0:64],
                    start=(tt == 0),
                    stop=(par == 1),
                    skip_group_check=True,
                )
            # M += K_pair^T V_pair: one K=128,N=128 matmul per head pair;
            # cross 64x64 blocks hold garbage, masked out at snapshot time.
            for pr in range(2):
                nc.tensor.matmul(
                    pm_t[:, ts(pr, P)],
                    lhsT=kn[:, DH * tt + P * pr :][:, 0:P],
                    rhs=vkn[:, DH * tt + P * pr :][:, 0:P],
                    start=(tt == 0 and pr == 0),
                    stop=(pr == 1),
                    skip_group_check=True,
                )
            # x = spike(scale * O).  O is integer, so for power-of-two
            # scale this is exactly (O >= 1/scale): one DVE op straight
            # from PSUM into the f32r xs tile.  Otherwise fall back to the
            # exact relu(1 - scale*O) <= 0 two-op chain.
            for idx in range(2):
                if pow2_scale:
                    nc.vector.tensor_scalar(
                        xs[:, 256 * tt + 128 * idx :][:, 0:P],
                        o_ps[idx][:, :],
                        float(1.0 / scale),
                        None,
                        ALU.is_ge,
                    )
                else:
                    xtmp = t_pool.tile([P, P], f32, name="xtmp")
                    nc.scalar.activation(
                        xtmp[:, :], o_ps[idx][:, :], AF.Relu,
                        bias=1.0, scale=-float(scale),
                    )
                    nc.vector.tensor_scalar(
                        xs[:, 256 * tt + 128 * idx :][:, 0:P],
                        xtmp[:, :],
                        0.0,
                        None,
                        ALU.is_le,
                    )

        def proj_piece(pc):
            ks_chunk(pc)
            for tt in range(4 * pc, 4 * pc + 4):
                vkn_block(tt)
        # Final projection per piece: output rows r with r%4 == m contract
        # only over attention piece m (X[r, f] = x_att[t=512*(r%4)+f,
        # d=r//4]).  A head pair's 128 rows are one contiguous xs block.

        def final_acc(yps, m, cc):
            for j in range(2):
                lhsT, rhs = mm_ops(
                    "o",
                    xs[:, 256 * (4 * m + cc) + 128 * j :][:, 0:P],
                    wo_sb[cc][:, :],
                )
                nc.tensor.matmul(
                    yps[j][:, :],
                    lhsT=lhsT,
                    rhs=rhs,
                    start=(cc == 0),
                    stop=(cc == KC - 1) and not has_bo,
                )

        def final_finish(yps, m):
            for j in range(2):  # head pair: heads 2j, 2j+1
                if has_bo:
                    nc.tensor.matmul(
                        yps[j][:, :],
                        lhsT=ones_sb[0:1, 0:P],
                        rhs=bias_sb[2:3, :],
                        start=False,
                        stop=True,
                    )
                y_sb = y_pool.tile([P, D], f16, name="y_sb")
                nc.vector.tensor_scalar(
                    y_sb[:, :], yps[j][:, :], 1.0, None, ALU.is_ge
                )
                nc.gpsimd.dma_start(out=y[2 * m + j], in_=y_sb[:, :])

        def final_piece(m):
            yps = [pp.tile([P, 512], f32, name="pt", tag="pt") for _ in range(2)]
            for cc in range(KC):
                final_acc(yps, m, cc)
            final_finish(yps, m)

        # Emission order tuned so the Tensor queue never stalls on a
        # transfer that is still behind others in a DMA queue: piece-0
        # work (smallest data prefix) first, attention starts before the
        # remaining qs chunks, wo arrives (Scalar queue) by final_piece(0).
        qproj([0])
        proj_piece(0)
        for tt in range(0, 4):
            attn_block(tt)
        qproj([1])
        proj_piece(1)
        final_piece(0)
        qproj([2])
        proj_piece(2)
        for tt in range(4, 8):
            attn_block(tt)
        final_piece(1)
        qproj([3])
        proj_piece(3)
        # pieces 2/3: no projection work remains, so the pp "pt" bufs are
        # free to hold the final-projection accumulators across the
        # attention blocks -- only spike+store remain after the last block.
        yps2 = [pp.tile([P, 512], f32, name="pt", tag="pt") for _ in range(2)]
        for tt in range(8, 12):
            attn_block(tt)
            final_acc(yps2, 2, tt % 4)
        final_finish(yps2, 2)
        yps3 = [pp.tile([P, 512], f32, name="pt", tag="pt") for _ in range(2)]
        for tt in range(12, 16):
            attn_block(tt)
            final_acc(yps3, 3, tt % 4)
        final_finish(yps3, 3)

    nc.compile()
    return nc


def _get_prog(scale, has_bk, has_bv, has_bo):
    key = (scale, has_bk, has_bv, has_bo, tuple(sorted(MODES.items())))
    if key not in _prog_cache:
        _prog_cache[key] = _build(scale, has_bk, has_bv, has_bo, MODES)
    return _prog_cache[key]


def _rne12(x):
    """Round fp32 -> float32r (11 explicit mantissa bits, RNE)."""
    u = np.ascontiguousarray(x, dtype=np.float32).view(np.uint32).astype(np.uint64)
    lsb = (u >> 12) & 1
    u = (u + 0x7FF + lsb) & 0xFFFFF000
    return u.astype(np.uint32).view(np.float32)


def _pack_weights(Wq, bq, Wk, bk, Wv, bv, Wo, bo, cs):
    r = lambda which, x: _rne12(x) if MODES[which] == "r" else np.float32(x)
    wpk = np.zeros((P, WPACK_W), np.float32)
    wpk[:DIN, OFF_WQ : OFF_WQ + DH] = r("q", Wq[:, cs])
    wpk[DIN, OFF_WQ : OFF_WQ + DH] = r("q", bq[cs])
    wpk[:, OFF_MSK : OFF_MSK + DH] = np.tile(
        np.triu(np.ones((P, P), np.float32)), (1, 2)
    )
    bdg = np.zeros((P, DH), np.float32)
    for pr in range(2):
        for par in range(2):
            sl = slice(64 * par, 64 * par + 64)
            bdg[sl, 128 * pr + 64 * par : 128 * pr + 64 * par + 64] = 1.0
    wpk[:, OFF_BDG : OFF_BDG + DH] = bdg
    for c in range(KC):
        wpk[:, OFF_WK + 256 * c : OFF_WK + 256 * (c + 1)] = r(
            "k", Wk[128 * c : 128 * (c + 1), cs]
        )
        wpk[:, OFF_WV + 256 * c : OFF_WV + 256 * (c + 1)] = r(
            "v", Wv[128 * c : 128 * (c + 1), cs]
        )
        wpk[:, OFF_WO + 512 * c : OFF_WO + 512 * (c + 1)] = r(
            "o", Wo[128 * c : 128 * (c + 1), :]
        )
    wpk[0, OFF_BIAS : OFF_BIAS + DH] = r("k", bk[cs])
    wpk[1, OFF_BIAS : OFF_BIAS + DH] = r("v", bv[cs])
    wpk[2, OFF_BIAS : OFF_BIAS + D] = r("o", bo)
    return wpk


def kernel(**inputs) -> np.ndarray:
    global last_exec_time_ns
    from concourse.bass_utils import run_bass_kernel_spmd

    g = lambda n: np.asarray(inputs[n], dtype=np.float32)
    query, key, value = g("query"), g("key"), g("value")
    Wq, bq, Wk, bk = g("Wq"), g("bq"), g("Wk"), g("bk")
    Wv, bv, Wo, bo = g("Wv"), g("bv"), g("Wo"), g("bo")
    scale = float(np.asarray(inputs["scale"], dtype=np.float32).reshape(-1)[0])

    has_bk, has_bv, has_bo = (bool(np.any(x)) for x in (bk, bv, bo))
    prog = _get_prog(scale, has_bk, has_bv, has_bo)

    rd = lambda which, x: _rne12(x) if MODES[which] == "r" else np.ascontiguousarray(x, np.float32)
    in_maps = []
    for c in range(NCORES):
        b, hg = divmod(c, 2)
        cs = slice(DH * hg, DH * (hg + 1))
        qTa = np.zeros((P, T), np.float32)
        qTa[:DIN] = rd("q", query[b].T)
        qTa[DIN] = 1.0
        in_maps.append(
            {
                "qT": qTa,
                "kT": rd("k", key[b].T),
                "vT": rd("v", value[b].T),
                "wpk": _pack_weights(Wq, bq, Wk, bk, Wv, bv, Wo, bo, cs),
            }
        )

    trace = os.environ.get("BASS_TRACE", "") not in ("", "0")
    res = run_bass_kernel_spmd(
        prog, in_maps, core_ids=list(range(NCORES)), trace=trace
    )
    last_exec_time_ns = res.exec_time_ns
    if res.exec_time_ns is not None:
        print(f"HW exec time: {res.exec_time_ns} ns")

    # y[2m+j, 64*sub + i, :] -> full row 256*(2j+sub) + m + 4i
    mi, pi = np.meshgrid(np.arange(2 * NPIECE), np.arange(P), indexing="ij")
    m, j, sub, i = mi // 2, mi % 2, pi // 64, pi % 64
    rows = (256 * (2 * j + sub) + m + 4 * i).ravel()
    inv = np.empty(1024, np.int64)
    inv[rows] = np.arange(1024)
    out = np.empty((B, T, D), np.float32)
    for c in range(NCORES):
        b, hg = divmod(c, 2)
        yc = res.results[c]["y"].reshape(1024, D)
        out[b, 1024 * hg : 1024 * (hg + 1)] = yc[inv].astype(np.float32)
    return out


# revision 59
# speedup vs baseline: 1.1200x; 1.1020x over previous
"""Trainium2 Bass kernel: spiking multi-head attention (nn_MultiHeadedAttention).

Reference semantics (B=4, T=2048, DIN=100, D=512, h=8 heads, dk=64):
    q = spike(query @ Wq + bq)   (spike = (x >= 1.0) -> {0,1})
    k = spike(key @ Wk + bk);  v = spike(value @ Wv + bv)
    attn = (q @ k^T) * scale, causally masked (keep k<=q), NO softmax
    x = spike(attn @ v)
    x = x.transpose(0,1,3,2).reshape(B,T,h*dk)    # scrambled reshape
    y = spike(x @ Wo + bo)

Key facts exploited:
  * No softmax -> causal attention is LINEAR attention:
        O_t = q_t . M_t  +  intra-block tril(Q K^T) V,   M = sum_j k_j v_j^T
    The running 64x64/head state M accumulates in PSUM across 16 t-blocks.
  * The scrambled reshape maps output rows [256*h, 256*(h+1)) to exactly one
    head h, so head-parallel sharding needs NO cross-core communication.
  * Spiked tensors are {0,1}; fp16 matmuls (1 PE pass) are bit-exact for them.
  * fp32 matmuls cost 2 PE passes, each emitted as its own ~592ns
    instruction.  float32r (fp32 with the low 12 mantissa bits zeroed,
    tf32-like) runs ONE pass when the moving dim is >=256 and the hardware
    computes the exact product of the rounded operands.  All four dense
    projections (q/k/v/final) run in f32r with host-side RNE rounding of
    data+weights; the final projection's moving operand {0,1} is exact.
  * DMA issues cost ~0.6us on the issuing engine; the baseline serialized
    38 issues on Sync (~45us of dead PE at the front).  v2 issues 13 big
    transfers across the Sync/Scalar/GpSimd queues, ordered so wq/wk/kt
    piece 0 land first.

Sharding: core c -> batch b=c//2, head-group hg=c%2 (4 heads per core).

Hardware pitfalls encoded below:
  * K=64 matmuls at partition base 0 vs 64 run concurrently in disjoint PE
    row groups; concurrent writes to one PSUM bank hang the device, so the
    two parity S-tiles live in separate banks.
  * start=True zeroes a whole 2KB PSUM bank region; PSUM allocation is
    bank-granular so every tile owns its bank.
  * GPSIMD cannot read PSUM; the masked M snapshot runs on Vector.
  * f32r matmul inputs must be produced by instructions whose output dtype
    is float32r (bir verifier) -- DMA into f32r tiles and DVE f32r stores
    both qualify.
"""

import os
import numpy as np

B, T, DIN, D = 4, 2048, 100, 512
H, DK = 8, 64
NCORES = 8
HPC = 4          # heads per core
DH = HPC * DK    # 256 projected features per core
P = 128
NT = T // P      # 16 t-blocks
KC = D // P      # 4 contraction chunks of the D=512 dim
NPIECE = 4       # pipeline pieces along T

# packed-weights column offsets (4-byte columns of the [128, WPACK_W] tensor)
OFF_WQ = 0
OFF_MSK = 256
OFF_BDG = 512
OFF_WK = 768
OFF_WV = 1792
OFF_WO = 2816
OFF_BIAS = 4864
WPACK_W = 5376

_prog_cache: dict = {}
last_exec_time_ns = None

# per-projection precision: 'r' = float32r (1 PE pass), 'f' = fp32 (2 passes)
MODES = {"q": "r", "k": "r", "v": "r", "o": "r"}


def _build(scale: float, has_bk: bool, has_bv: bool, has_bo: bool, modes: dict):
    from contextlib import ExitStack

    import concourse.bass as bass
    import concourse.tile as tile
    import concourse.mybir as mybir
    from concourse import bacc
    from concourse.bass import ts
    from concourse import masks

    f32 = mybir.dt.float32
    f32r = mybir.dt.float32r
    f16 = mybir.dt.float16
    ALU = mybir.AluOpType
    AF = mybir.ActivationFunctionType
    BIG = float(2 ** 26)
    import math

    pow2_scale = scale > 0 and math.frexp(scale)[0] == 0.5

    nc = bacc.Bacc(
        "TRN2", target_bir_lowering=False, debug=False, num_devices=NCORES
    )

    # DRAM I/O.  All dense-projection operands are declared float32r; a
    # projection running in fp32 mode just bitcasts its views back to f32
    # (the host then skips rounding those sections).
    qT = nc.dram_tensor("qT", [P, T], f32r, kind="ExternalInput").ap()
    kT = nc.dram_tensor("kT", [D, T], f32r, kind="ExternalInput").ap()
    vT = nc.dram_tensor("vT", [D, T], f32r, kind="ExternalInput").ap()
    wpk = nc.dram_tensor("wpk", [P, WPACK_W], f32r, kind="ExternalInput").ap()
    # y[2m+j] = final spike block for piece m, head pair j (contiguous
    # stores; the host unscrambles the row interleave).
    y = nc.dram_tensor("y", [2 * NPIECE, P, D], f16, kind="ExternalOutput").ap()

    def mm_ops(which, lhsT, rhs):
        if modes[which] == "r":
            return lhsT, rhs
        return lhsT.bitcast(f32), rhs.bitcast(f32)

    with tile.TileContext(nc) as tc, ExitStack() as ctx:
        pool = lambda name, bufs, space="SBUF": ctx.enter_context(
            tc.tile_pool(name=name, bufs=bufs, space=space)
        )
        persist = pool("persist", 1)      # distinct tags -> own slots
        s_pool = pool("s_pool", 4)        # masked S tiles (fp16)
        t_pool = pool("t_pool", 4)        # ACT-chain temporaries
        m_pool = pool("m_pool", 2)        # masked M snapshots
        y_pool = pool("y_pool", 3)        # output staging
        pp = pool("pp", 3, "PSUM")        # projections/final/transposes
        ps = pool("ps", 1, "PSUM")        # S^T tiles (2 parity tags)
        po = pool("po", 2, "PSUM")        # O pair accumulators
        pm = pool("pm", 1, "PSUM")        # persistent M state

        def ptile(shape, dtype=f32, *, name):
            return persist.tile(shape, dtype, name=name, tag=name)

        # ---- SBUF allocations -----------------------------------------
        # Every independently-loaded region gets its OWN tile: the tile
        # framework chains DMAs writing one tile (WAW) with a ~2us
        # semaphore round-trip per link, so shared tiles serialize the
        # whole input stream.
        qt_sb = ptile([P, T], f32r, name="qt_sb")
        kt_sb = [ptile([P, T], f32r, name=f"kt_sb{c}") for c in range(KC)]
        vt_sb = [ptile([P, T], f32r, name=f"vt_sb{c}") for c in range(KC)]
        wq_t = ptile([P, DH], f32r, name="wq_t")
        mb_t = ptile([P, 2 * DH], f32r, name="mb_t")
        wk_t = ptile([P, KC * DH], f32r, name="wk_t")
        wv_t = ptile([P, KC * DH], f32r, name="wv_t")
        wob_t = ptile([P, KC * D + D], f32r, name="wob_t")
        wq_sb = wq_t[:, :]
        msk_sb = mb_t[:, 0:DH].bitcast(f32)
        bdg_sb = mb_t[:, DH : 2 * DH].bitcast(f32)
        wk_sb = [wk_t[:, 256 * c : 256 * (c + 1)] for c in range(KC)]
        wv_sb = [wv_t[:, 256 * c : 256 * (c + 1)] for c in range(KC)]
        wo_sb = [wob_t[:, 512 * c : 512 * (c + 1)] for c in range(KC)]
        bias_sb = wob_t[:, KC * D : KC * D + D]
        ones_sb = ptile([1, D], f32r, name="ones_sb")
        idt_sb = ptile([P, P], f16, name="idt_sb")
        # qs/ks: spiked projections, d-major [dk, T]; tile i holds heads
        # 2i (parts 0:64) and 2i+1 (parts 64:128).  fp16: {0,1} and the
        # integer M state (<= 2048 < 2^11) are exact, 1 PE pass.
        qs = [ptile([P, T], f16, name=f"qs{i}") for i in range(2)]
        ks = [ptile([P, T], f16, name=f"ks{i}") for i in range(2)]
        # vkn: t-major spiked v for all 4 heads (cols 256t+64*hl), fp16.
        vkn = ptile([P, DH * NT], f16, name="vkn")
        # kn: t-major spiked k via PE transpose of ks, pair-major:
        # cols 256t + 128*pair + 64*(hl%2)
        kn = ptile([P, DH * NT], f16, name="kn")
        # xs: spiked attention output, xs[p, 256*t_blk + 128*pair + 64*par
        # + d]; f32r so the final projection consumes it in one PE pass
        # ({0,1} exact), contiguous per (t_blk, pair) for both the DVE
        # store and the final-proj weight load.
        xs = ptile([P, 256 * NT], f32r, name="xs")

        # ---- loads ----------------------------------------------------
        # Sync carries the kproj critical path (kt piece 0 per chunk, wk,
        # then qT), Scalar carries wv/vt piece 0 + wo, GpSimd the vt bulk.
        # Distinct dst tiles keep every queue's transfers streaming
        # back-to-back with no cross-transfer semaphore links.
        PW = T // NPIECE
        # Arrival-deadline schedule.  Big 3D piece transfers (all four
        # 128-row chunks in one ~1MB issue) beat per-chunk 0.25MB issues
        # (~1.3us fixed cost each).  Sync carries the k-side critical
        # path; Scalar only 4 early issues (its ACT work starts ~13us);
        # the vt bulk rides GpSimd behind a gate-copy so it cannot steal
        # ring bandwidth from the prefix.  make_identity is emitted
        # before the gate so the transposes' identity tile exists early.
        nc.sync.dma_start(
            out=wq_t[:, :], in_=wpk[:, OFF_WQ : OFF_WQ + DH]
        )
        nc.sync.dma_start(out=wk_t[:, :], in_=wpk[:, OFF_WK:OFF_WV])
        nc.sync.dma_start(out=kt_sb[0][:, 0:PW], in_=kT[ts(0, P), 0:PW])
        nc.sync.dma_start(out=kt_sb[2][:, 0:PW], in_=kT[ts(2, P), 0:PW])
        nc.sync.dma_start(out=mb_t[:, :], in_=wpk[:, OFF_MSK:OFF_WK])
        nc.sync.dma_start(out=wob_t[:, :], in_=wpk[:, OFF_WO:WPACK_W])
        nc.scalar.dma_start(out=qt_sb[:, 0:512], in_=qT[:, 0:512])
        nc.scalar.dma_start(out=kt_sb[1][:, 0:PW], in_=kT[ts(1, P), 0:PW])
        nc.scalar.dma_start(out=wv_t[:, :], in_=wpk[:, OFF_WV:OFF_WO])
        nc.scalar.dma_start(out=kt_sb[3][:, 0:PW], in_=kT[ts(3, P), 0:PW])
        for c in range(KC):
            nc.scalar.dma_start(out=vt_sb[c][:, 0:PW], in_=vT[ts(c, P), 0:PW])
        nc.scalar.dma_start(out=qt_sb[:, 512:T], in_=qT[:, 512:T])
        nc.vector.memset(ones_sb[:, :].bitcast(f32), 1.0)
        masks.make_identity(nc, idt_sb[:, :])
        # gate: the copy reads vt piece 0 (RAW), so later GpSimd
        # instructions (FIFO) wait for the prefix before the bulk pull.
        # Emitted after make_identity so the transposes' identity tile is
        # built before the gate blocks the GpSimd queue.
        gate_sb = ptile([1, 1], f32, name="gate_sb")
        nc.gpsimd.tensor_copy(
            gate_sb[:, :], vt_sb[KC - 1][0:1, PW - 1 : PW].bitcast(f32)
        )
        for pc in range(1, NPIECE):
            for c in range(KC):
                nc.gpsimd.dma_start(
                    out=kt_sb[c][:, ts(pc, PW)], in_=kT[ts(c, P), ts(pc, PW)]
                )
                nc.gpsimd.dma_start(
                    out=vt_sb[c][:, ts(pc, PW)], in_=vT[ts(c, P), ts(pc, PW)]
                )

        def spike_act(out_ap, in_ap, nm):
            """out = (in >= 1.0) via two exact Relu ops on the ACT engine."""
            tmp = t_pool.tile(list(out_ap.shape), f32, name=f"tmp_{nm}")
            nc.scalar.activation(tmp[:, :], in_ap, AF.Relu, bias=1.0, scale=-1.0)
            nc.scalar.activation(out_ap, tmp[:, :], AF.Relu, bias=1.0, scale=-BIG)

        # ---- qs projection (only needs qt + wq) ------------------------
        def qproj(chunks):
            for ch in chunks:
                for half in range(2):
                    pt = pp.tile([P, 512], f32, name="pt", tag="pt")
                    lhsT, rhs = mm_ops(
                        "q", wq_sb[:, ts(half, P)], qt_sb[:, ts(ch, 512)]
                    )
                    nc.tensor.matmul(
                        pt[:, :], lhsT=lhsT, rhs=rhs, start=True, stop=True
                    )
                    spike_act(qs[half][:, ts(ch, 512)], pt[:, :], "q")

        # ---- pipelined: per piece, ks chunk -> vkn blocks -> attention -
        pm_t = pm.tile([P, DH], f32, name="pm_t")

        def ks_chunk(ch):
            for half in range(2):
                pt = pp.tile([P, 512], f32, name="pt", tag="pt")
                for c in range(KC):
                    lhsT, rhs = mm_ops(
                        "k", wk_sb[c][:, ts(half, P)], kt_sb[c][:, ts(ch, 512)]
                    )
                    nc.tensor.matmul(
                        pt[:, :],
                        lhsT=lhsT,
                        rhs=rhs,
                        start=(c == 0),
                        stop=(c == KC - 1) and not has_bk,
                    )
                if has_bk:
                    nc.tensor.matmul(
                        pt[:, :],
                        lhsT=bias_sb[0:1, ts(half, P)],
                        rhs=ones_sb[0:1, 0:512],
                        start=False,
                        stop=True,
                    )
                spike_act(ks[half][:, ts(ch, 512)], pt[:, :], "k")
            # t-major spiked K for this chunk's 4 blocks via PE transpose;
            # a [128,128] head-pair tile transpose lands exactly in the
            # pair-major layout the M-update wants.  (A DMA-xbar transpose
            # is bit-exact in isolation but showed ~100 extra spike flips
            # when overlapped with the input loads, so it stays on the PE.)
            for tt in range(4 * ch, 4 * ch + 4):
                for pr in range(2):
                    tp = pp.tile([P, P], f16, name="tp", tag="pt")
                    nc.tensor.transpose(
                        tp[:, :], ks[pr][:, ts(tt, P)], idt_sb[:, :]
                    )
                    nc.vector.tensor_copy(
                        kn[:, DH * tt + P * pr :][:, 0:P], tp[:, :]
                    )

        def vkn_block(tt):
            pt = pp.tile([P, 512], f32, name="pt", tag="pt")
            for c in range(KC):
                lhsT, rhs = mm_ops(
                    "v", vt_sb[c][:, ts(tt, P)], wv_sb[c][:, :]
                )
                nc.tensor.matmul(
                    pt[:, 0:DH],
                    lhsT=lhsT,
                    rhs=rhs,
                    start=(c == 0),
                    stop=(c == KC - 1) and not has_bv,
                )
            if has_bv:
                nc.tensor.matmul(
                    pt[:, 0:DH],
                    lhsT=ones_sb[0:1, 0:P],
                    rhs=bias_sb[1:2, 0:DH],
                    start=False,
                    stop=True,
                )
            nc.vector.tensor_scalar(
                vkn[:, ts(tt, DH)], pt[:, 0:DH], 1.0, None, ALU.is_ge
            )

        def attn_block(tt):
            if tt > 0:
                # masked snapshot M_(<tt): zero the cross-head 64x64 blocks
                # so the pair O-inter matmul can contract over all 128
                # partition rows at once.
                m_sb = m_pool.tile([P, DH], f16, name="m_sb")
                nc.vector.tensor_tensor(
                    m_sb[:, :], pm_t[:, :], bdg_sb[:, :], op=ALU.mult
                )
            s_ps = [
                ps.tile([P, DH], f32, name=f"s_ps{par}", tag=f"s_ps{par}")
                for par in range(2)
            ]
            for hl in range(HPC):
                par, idx = hl % 2, hl // 2
                rows = slice(64 * par, 64 * par + 64)
                nc.tensor.matmul(
                    s_ps[par][:, ts(idx, P)],
                    lhsT=ks[idx][rows, ts(tt, P)],
                    rhs=qs[idx][rows, ts(tt, P)],
                    start=True,
                    stop=True,
                )
            s_sb = [
                s_pool.tile([P, DH], f16, name=f"s_sb{par}", tag=f"s_sb{par}")
                for par in range(2)
            ]
            for par in range(2):
                nc.vector.tensor_tensor(
                    s_sb[par][:, :], s_ps[par][:, :], msk_sb[:, :], op=ALU.mult
                )
            # O pair tiles: cols 0:64 head 2*idx, 64:128 head 2*idx+1.
            o_ps = [po.tile([P, P], f32, name="o_ps") for _ in range(2)]
            for idx in range(2):
                if tt > 0:
                    nc.tensor.matmul(
                        o_ps[idx][:, :],
                        lhsT=qs[idx][:, ts(tt, P)],
                        rhs=m_sb[:, ts(idx, P)],
                        start=True,
                        stop=False,
                        skip_group_check=True,
                    )
            for hl in range(HPC):
                par, idx = hl % 2, hl // 2
                nc.tensor.matmul(
                    o_ps[idx][:, ts(par, 64)],
                    lhsT=s_sb[par][:, ts(idx, P)],
                    rhs=vkn[:, DH * tt + 64 * hl :][:, 0:64],
                    start=(tt == 0),
                    stop=(par == 1),
                    skip_group_check=True,
                )
            # M += K_pair^T V_pair: one K=128,N=128 matmul per head pair;
            # cross 64x64 blocks hold garbage, masked out at snapshot time.
            for pr in range(2):
                nc.tensor.matmul(
                    pm_t[:, ts(pr, P)],
                    lhsT=kn[:, DH * tt + P * pr :][:, 0:P],
                    rhs=vkn[:, DH * tt + P * pr :][:, 0:P],
                    start=(tt == 0 and pr == 0),
                    stop=(pr == 1),
                    skip_group_check=True,
                )
            # x = spike(scale * O).  O is integer, so for power-of-two
            # scale this is exactly (O >= 1/scale): one DVE op straight
            # from PSUM into the f32r xs tile.  Otherwise fall back to the
            # exact relu(1 - scale*O) <= 0 two-op chain.
            for idx in range(2):
                if pow2_scale:
                    nc.vector.tensor_scalar(
                        xs[:, 256 * tt + 128 * idx :][:, 0:P],
                        o_ps[idx][:, :],
                        float(1.0 / scale),
                        None,
                        ALU.is_ge,
                    )
                else:
                    xtmp = t_pool.tile([P, P], f32, name="xtmp")
                    nc.scalar.activation(
                        xtmp[:, :], o_ps[idx][:, :], AF.Relu,
                        bias=1.0, scale=-float(scale),
                    )
                    nc.vector.tensor_scalar(
                        xs[:, 256 * tt + 128 * idx :][:, 0:P],
                        xtmp[:, :],
                        0.0,
                        None,
                        ALU.is_le,
                    )

        def proj_piece(pc):
            ks_chunk(pc)
            for tt in range(4 * pc, 4 * pc + 4):
                vkn_block(tt)
        # Final projection per piece: output rows r with r%4 == m contract
        # only over attention piece m (X[r, f] = x_att[t=512*(r%4)+f,
        # d=r//4]).  A head pair's 128 rows are one contiguous xs block.

        def final_acc(yps, m, cc):
            for j in range(2):
                lhsT, rhs = mm_ops(
                    "o",
                    xs[:, 256 * (4 * m + cc) + 128 * j :][:, 0:P],
                    wo_sb[cc][:, :],
                )
                nc.tensor.matmul(
                    yps[j][:, :],
                    lhsT=lhsT,
                    rhs=rhs,
                    start=(cc == 0),
                    stop=(cc == KC - 1) and not has_bo,
                )

        def final_finish(yps, m):
            for j in range(2):  # head pair: heads 2j, 2j+1
                if has_bo:
                    nc.tensor.matmul(
                        yps[j][:, :],
                        lhsT=ones_sb[0:1, 0:P],
                        rhs=bias_sb[2:3, :],
                        start=False,
                        stop=True,
                    )
                y_sb = y_pool.tile([P, D], f16, name="y_sb")
                nc.vector.tensor_scalar(
                    y_sb[:, :], yps[j][:, :], 1.0, None, ALU.is_ge
                )
                nc.gpsimd.dma_start(out=y[2 * m + j], in_=y_sb[:, :])

        def final_piece(m):
            yps = [pp.tile([P, 512], f32, name="pt", tag="pt") for _ in range(2)]
            for cc in range(KC):
                final_acc(yps, m, cc)
            final_finish(yps, m)

        # Emission order tuned so the Tensor queue never stalls on a
        # transfer that is still behind others in a DMA queue: piece-0
        # work (smallest data prefix) first, attention starts before the
        # remaining qs chunks, wo arrives (Scalar queue) by final_piece(0).
        qproj([0])
        proj_piece(0)
        for tt in range(0, 4):
            attn_block(tt)
        qproj([1])
        proj_piece(1)
        final_piece(0)
        qproj([2])
        proj_piece(2)
        for tt in range(4, 8):
            attn_block(tt)
        final_piece(1)
        qproj([3])
        proj_piece(3)
        # pieces 2/3: no projection work remains, so the pp "pt" bufs are
        # free to hold the final-projection accumulators across the
        # attention blocks -- only spike+store remain after the last block.
        yps2 = [pp.tile([P, 512], f32, name="pt", tag="pt") for _ in range(2)]
        for tt in range(8, 12):
            attn_block(tt)
            final_acc(yps2, 2, tt % 4)
        final_finish(yps2, 2)
        yps3 = [pp.tile([P, 512], f32, name="pt", tag="pt") for _ in range(2)]
        for tt in range(12, 16):
            attn_block(tt)
            final_acc(yps3, 3, tt % 4)
        final_finish(yps3, 3)

    nc.compile()
    return nc


def _get_prog(scale, has_bk, has_bv, has_bo):
    key = (scale, has_bk, has_bv, has_bo, tuple(sorted(MODES.items())))
    if key not in _prog_cache:
        _prog_cache[key] = _build(scale, has_bk, has_bv, has_bo, MODES)
    return _prog_cache[key]


def _rne12(x):
    """Round fp32 -> float32r (11 explicit mantissa bits, RNE)."""
    u = np.ascontiguousarray(x, dtype=np.float32).view(np.uint32).astype(np.uint64)
    lsb = (u >> 12) & 1
    u = (u + 0x7FF + lsb) & 0xFFFFF000
    return u.astype(np.uint32).view(np.float32)


def _pack_weights(Wq, bq, Wk, bk, Wv, bv, Wo, bo, cs):
    r = lambda which, x: _rne12(x) if MODES[which] == "r" else np.float32(x)
    wpk = np.zeros((P, WPACK_W), np.float32)
    wpk[:DIN, OFF_WQ : OFF_WQ + DH] = r("q", Wq[:, cs])
    wpk[DIN, OFF_WQ : OFF_WQ + DH] = r("q", bq[cs])
    wpk[:, OFF_MSK : OFF_MSK + DH] = np.tile(
        np.triu(np.ones((P, P), np.float32)), (1, 2)
    )
    bdg = np.zeros((P, DH), np.float32)
    for pr in range(2):
        for par in range(2):
            sl = slice(64 * par, 64 * par + 64)
            bdg[sl, 128 * pr + 64 * par : 128 * pr + 64 * par + 64] = 1.0
    wpk[:, OFF_BDG : OFF_BDG + DH] = bdg
    for c in range(KC):
        wpk[:, OFF_WK + 256 * c : OFF_WK + 256 * (c + 1)] = r(
            "k", Wk[128 * c : 128 * (c + 1), cs]
        )
        wpk[:, OFF_WV + 256 * c : OFF_WV + 256 * (c + 1)] = r(
            "v", Wv[128 * c : 128 * (c + 1), cs]
        )
        wpk[:, OFF_WO + 512 * c : OFF_WO + 512 * (c + 1)] = r(
            "o", Wo[128 * c : 128 * (c + 1), :]
        )
    wpk[0, OFF_BIAS : OFF_BIAS + DH] = r("k", bk[cs])
    wpk[1, OFF_BIAS : OFF_BIAS + DH] = r("v", bv[cs])
    wpk[2, OFF_BIAS : OFF_BIAS + D] = r("o", bo)
    return wpk


def kernel(**inputs) -> np.ndarray:
    global last_exec_time_ns
    from concourse.bass_utils import run_bass_kernel_spmd

    g = lambda n: np.asarray(inputs[n], dtype=np.float32)
    query, key, value = g("query"), g("key"), g("value")
    Wq, bq, Wk, bk = g("Wq"), g("bq"), g("Wk"), g("bk")
    Wv, bv, Wo, bo = g("Wv"), g("bv"), g("Wo"), g("bo")
    scale = float(np.asarray(inputs["scale"], dtype=np.float32).reshape(-1)[0])

    has_bk, has_bv, has_bo = (bool(np.any(x)) for x in (bk, bv, bo))
    prog = _get_prog(scale, has_bk, has_bv, has_bo)

    rd = lambda which, x: _rne12(x) if MODES[which] == "r" else np.ascontiguousarray(x, np.float32)
    in_maps = []
    for c in range(NCORES):
        b, hg = divmod(c, 2)
        cs = slice(DH * hg, DH * (hg + 1))
        qTa = np.zeros((P, T), np.float32)
        qTa[:DIN] = rd("q", query[b].T)
        qTa[DIN] = 1.0
        in_maps.append(
            {
                "qT": qTa,
                "kT": rd("k", key[b].T),
                "vT": rd("v", value[b].T),
                "wpk": _pack_weights(Wq, bq, Wk, bk, Wv, bv, Wo, bo, cs),
            }
        )

    trace = os.environ.get("BASS_TRACE", "") not in ("", "0")
    res = run_bass_kernel_spmd(
        prog, in_maps, core_ids=list(range(NCORES)), trace=trace
    )
    last_exec_time_ns = res.exec_time_ns
    if res.exec_time_ns is not None:
        print(f"HW exec time: {res.exec_time_ns} ns")

    # y[2m+j, 64*sub + i, :] -> full row 256*(2j+sub) + m + 4i
    mi, pi = np.meshgrid(np.arange(2 * NPIECE), np.arange(P), indexing="ij")
    m, j, sub, i = mi // 2, mi % 2, pi // 64, pi % 64
    rows = (256 * (2 * j + sub) + m + 4 * i).ravel()
    inv = np.empty(1024, np.int64)
    inv[rows] = np.arange(1024)
    out = np.empty((B, T, D), np.float32)
    for c in range(NCORES):
        b, hg = divmod(c, 2)
        yc = res.results[c]["y"].reshape(1024, D)
        out[b, 1024 * hg : 1024 * (hg + 1)] = yc[inv].astype(np.float32)
    return out
